# revision 35
# baseline (speedup 1.0000x reference)
"""Trainium2 kernel for nn_DiffClusterMIST (KNN mutual-information estimator).

Math: for each row i of X [8192, 256] with class label y_i:
  d2_ij = |x_i|^2 + |x_j|^2 - 2 x_i.x_j            (squared distance)
  anchor_i = 4th-smallest d2 among same-class j (self included)
  m_i = #{j: d2_ij < anchor_i} - 1                 (over ALL j)
  mi = (psi(N) - sum_c (N_c/N) psi(N_c) + psi(3) - mean_i psi(m_i + 1e-7)) / ln 2
  out = relu(mi)

Device trick: let u_ij = x_i.x_j - |x_j|^2/2. Then d2_ij = |x_i|^2 - 2 u_ij,
so d2-ordering/comparisons within a row are reversed u-orderings - no sqrt,
no per-row |x_i|^2 term, no clamp needed. Class masking is folded into the
matmul: append kappa*onehot(y) features to both operands (adds kappa^2 to
same-class u), plus a (ones, -|x_j|^2/2) feature pair so the matmul directly
produces u'_ij = u_ij + kappa^2*[y_i=y_j] in PSUM:
  A_i = [x_i, kappa*oh(y_i), 1]  (lhsT side),  B_j = [x_j, kappa*oh(y_j), -h_j]
With kappa^2 >> range(u), the row-wise top-4 of u' are exactly the 4 nearest
same-class points; threshold T' = max8(u')[3] - kappa^2 counts all-j strictly
above the anchor:  C_all = n_same(y_i) + #{diff j above}  =>
  m_i = 2 + C_all - n_same(y_i).

Sharding: rows split across the 8 cores (1024 rows each); B replicated.
Each core: 8 row-tiles x 16 col-chunks of fp32r matmul (K=267) -> PSUM,
ScalarE evacuates PSUM->SBUF, DVE max8 per row-tile, then the count is
column-split between DVE (tensor_scalar is_gt + accum) and ScalarE
(Sign activation + accum). Host finishes with exact digammas.
"""
import sys

if "/opt/trn_rl_repo" not in sys.path:
    sys.path.insert(0, "/opt/trn_rl_repo")

import numpy as np

N = 8192
D = 256
C = 10
KNN = 3
KAPPA = 256.0
KAPPA2 = KAPPA * KAPPA
NCORES = 8
ROWS_PER_CORE = N // NCORES          # 1024
RT = ROWS_PER_CORE // 128            # 8 row-tiles per core
KDIM = D + C + 1                     # 267
KX = C + 2                           # packed extra features: onehot, -h hi/lo
CHUNK = 512                          # matmul free dim (one PSUM bank)
GROUP = 2048                         # evac group (4 banks)
NGROUP = N // GROUP                  # 4
DVE_COLS = 7680
DVE_COLS_LAST = 4096                 # balance the post-matmul tail                    # count cols on DVE; rest on ScalarE
ACT_COLS = N - DVE_COLS
WIN = 2048                           # runtime-windowed max8 scan width

_PROGRAM = None
LAST_RESULTS = None


def _ensure_ntff_hook():
    """The agent image's `antenv` lacks `axon_hooks`, which bass_utils
    needs to capture NTFF profiles under axon (trace=True). Provide the
    module and register the ctypes-based hook from trn_agent_boot."""
    import types
    try:
        import antenv.axon_hooks  # noqa: F401
        return
    except ImportError:
        pass
    try:
        import antenv
        from trn_agent_boot.trn_boot import _ntff_profile_via_ctypes
        holder = [None]
        mod = types.ModuleType("antenv.axon_hooks")
        mod.set_axon_ntff_profile_hook = lambda h: holder.__setitem__(0, h)
        mod.get_axon_ntff_profile_hook = lambda: holder[0]
        sys.modules["antenv.axon_hooks"] = mod
        antenv.axon_hooks = mod
        mod.set_axon_ntff_profile_hook(
            _ntff_profile_via_ctypes("/opt/axon/libaxon_pjrt.so"))
    except Exception:
        pass


def _split_multi_waits(nc, mybir):
    """This walrus build allows one sync-wait per instruction; move extra
    waits onto preceding engine-local NoOps (equivalent: streams are
    in-order per engine)."""
    cnt = 0
    for f in nc.m.functions:
        for bb in f.blocks:
            out = []
            changed = False
            for inst in bb.instructions:
                si = inst.sync_info
                waits = list(si.on_wait) if si is not None else []
                if len(waits) > 1:
                    changed = True
                    for w in waits[:-1]:
                        cnt += 1
                        out.append(mybir.InstNoOp(
                            name=f"wsplit-{cnt}",
                            engine=inst.engine,
                            bass_nofuse=True,
                            sync_info=mybir.SyncInfo(on_wait=[w], on_update=[]),
                        ))
                    inst.sync_info = mybir.SyncInfo(
                        on_wait=[waits[-1]], on_update=list(si.on_update))
                out.append(inst)
            if changed:
                bb.instructions = out


def _build_program():
    import concourse.bass as bass
    import concourse.mybir as mybir
    from concourse.bass import ds
    from concourse.expressions import make_scalar_value
    from concourse.tile import TileContext

    f32 = mybir.dt.float32
    f32r = mybir.dt.float32r
    i8 = mybir.dt.int8

    nc = bass.Bass(trn_type="TRN2")
    bf16 = mybir.dt.bfloat16
    rhsB_d = nc.dram_tensor("rhsB", [D, N], bf16, kind="ExternalInput")
    lhsA_d = nc.dram_tensor("lhsA", [D, ROWS_PER_CORE], bf16, kind="ExternalInput")
    rhsB2_d = nc.dram_tensor("rhsB2", [KX, N], bf16, kind="ExternalInput")
    lhsA2_d = nc.dram_tensor("lhsA2", [KX, ROWS_PER_CORE], bf16, kind="ExternalInput")
    win_d = nc.dram_tensor("win", [1, RT], mybir.dt.int32, kind="ExternalInput")
    cnt_d = nc.dram_tensor("cnt", [128, RT], f32, kind="ExternalOutput")
    sgn_d = nc.dram_tensor("sgn", [128, RT], f32, kind="ExternalOutput")
    thr_d = nc.dram_tensor("thr", [128, RT], f32, kind="ExternalOutput")

    KT = [(0, 128), (128, 128)]  # X k-tiles; extra features packed in bf16

    with TileContext(nc) as tc:
        with tc.tile_pool(name="weights", bufs=1) as wpool, \
             tc.tile_pool(name="ubuf", bufs=3) as upool, \
             tc.tile_pool(name="junk", bufs=1) as jpool, \
             tc.tile_pool(name="small", bufs=3) as spool, \
             tc.tile_pool(name="psum", bufs=2, space="PSUM") as ppool:

            # A-block (lhsT) for this core's rows - on the gpsimd (SWDGE)
            # queue so it loads in parallel with the B panels below.
            # The 11 extra feature rows (onehot, ones/-h) are replicated at
            # partition offsets 0/32/64/96 so four chunks' K=11 matmuls can
            # run concurrently in disjoint PE row-groups (tile_position).
            lk = []
            for k, (ks, ksz) in enumerate(KT):
                t = wpool.tile([ksz, ROWS_PER_CORE], bf16, tag=f"lk{k}")
                nc.gpsimd.dma_start(t[:], lhsA_d[ks:ks + ksz, :])
                lk.append(t)
            lb2 = wpool.tile([64 + KX, ROWS_PER_CORE], bf16, tag="lb2")
            rb2 = wpool.tile([64 + KX, N], bf16, tag="rb2")
            for j in range(3):
                nc.gpsimd.dma_start(lb2[32 * j:32 * j + KX, :], lhsA2_d[:, :])
                nc.gpsimd.dma_start(rb2[32 * j:32 * j + KX, :], rhsB2_d[:, :])
            win_sb = wpool.tile([1, RT], mybir.dt.int32, tag="win")
            nc.gpsimd.dma_start(win_sb[:], win_d[:])
            # B (rhs) in 4 column panels per k-tile, loaded in consumption
            # order (panel-major) so the first matmuls start early
            rk = [[None] * NGROUP for _ in KT]
            for j in range(NGROUP):
                for k, (ks, ksz) in enumerate(KT):
                    t = wpool.tile([ksz, GROUP], bf16, tag=f"rk{k}_{j}")
                    nc.sync.dma_start(t[:], rhsB_d[ks:ks + ksz, j * GROUP:(j + 1) * GROUP])
                    rk[k][j] = t

            junkD = jpool.tile([128, DVE_COLS], i8, tag="junkD")
            junkA = jpool.tile([128, N - DVE_COLS_LAST], i8, tag="junkA")
            cntA = jpool.tile([128, RT], f32, tag="cntA")
            sgnA = jpool.tile([128, RT], f32, tag="sgnA")
            thrA = jpool.tile([128, RT], f32, tag="thrA")

            # Software pipeline: tile r's ScalarE sign-count is emitted
            # after tile r+1's matmul/evac section so the ScalarE stream
            # never stalls waiting on DVE's max8 of the same tile.
            pend = None

            def emit_counts(r, u, thr, negthr):
                dcols = DVE_COLS if r < RT - 1 else DVE_COLS_LAST
                nc.vector.tensor_scalar(
                    junkD[:, 0:dcols], u[:, 0:dcols], thr[:], None,
                    op0=mybir.AluOpType.is_gt, op1=mybir.AluOpType.add,
                    accum_out=cntA[:, r:r + 1])
                nc.scalar.activation(
                    junkA[:, 0:N - dcols], u[:, dcols:N],
                    mybir.ActivationFunctionType.Sign,
                    bias=negthr[:], accum_out=sgnA[:, r:r + 1])

            for r in range(RT):
                u = upool.tile([128, N], f32, tag="u")
                for g in range(NGROUP):
                    ps = ppool.tile([128, GROUP], f32, tag="ps")
                    # k-major: consecutive matmuls hit different PSUM banks,
                    # so each bank's accumulation-chain drain is hidden
                    for k in range(2):
                        for c in range(GROUP // CHUNK):
                            lo = c * CHUNK
                            nc.tensor.matmul(
                                ps[:, lo:lo + CHUNK],
                                lk[k][:, r * 128:(r + 1) * 128],
                                rk[k][g][:, lo:lo + CHUNK],
                                start=(k == 0), stop=False)
                    for c in range(GROUP // CHUNK):
                        lo = c * CHUNK
                        po = 32 * (c % 3)
                        nc.tensor.matmul(
                            ps[:, lo:lo + CHUNK],
                            lb2[po:po + KX, r * 128:(r + 1) * 128],
                            rb2[po:po + KX, g * GROUP + lo:g * GROUP + lo + CHUNK],
                            start=False, stop=True,
                            tile_position=(po, 0),
                            skip_group_check=True)
                    nc.scalar.activation(
                        u[:, g * GROUP:g * GROUP + 1024], ps[:, 0:1024],
                        mybir.ActivationFunctionType.Copy)
                    nc.scalar.activation(
                        u[:, g * GROUP + 1024:(g + 1) * GROUP], ps[:, 1024:2048],
                        mybir.ActivationFunctionType.Copy)
                    if g == 3 and pend is not None:
                        emit_counts(*pend)
                        pend = None

                m8 = spool.tile([128, 8], f32, tag="m8")
                wreg = nc.vector.alloc_register(f"win{r}")
                nc.vector.load(wreg, win_sb[0:1, r:r + 1])
                wsv = make_scalar_value(wreg, min_val=0, max_val=N - WIN)
                nc.vector.max(out=m8[:], in_=u[:, ds(wsv, WIN)])
                thr = thrA[:, r:r + 1]
                nc.vector.tensor_scalar_add(thr, m8[:, 3:4], -KAPPA2)
                negthr = spool.tile([128, 1], f32, tag="negthr")
                nc.vector.tensor_scalar_mul(negthr[:], thr, -1.0)
                pend = (r, u, thr, negthr)
            emit_counts(*pend)
            nc.sync.dma_start(cnt_d[:], cntA[:])
            nc.sync.dma_start(sgn_d[:], sgnA[:])
            nc.sync.dma_start(thr_d[:], thrA[:])

    _split_multi_waits(nc, mybir)
    return nc


def _digamma(x):
    """Vectorized digamma, float64, accurate for x > 0."""
    x = np.atleast_1d(np.asarray(x, dtype=np.float64)).copy()
    out = np.zeros_like(x)
    # recurrence psi(x) = psi(x+1) - 1/x until x >= 6
    for _ in range(8):
        mask = x < 6.0
        if not mask.any():
            break
        out[mask] -= 1.0 / x[mask]
        x[mask] += 1.0
    inv = 1.0 / x
    inv2 = inv * inv
    out += (np.log(x) - 0.5 * inv
            - inv2 * (1.0 / 12.0 - inv2 * (1.0 / 120.0 - inv2 * (1.0 / 252.0
                      - inv2 * (1.0 / 240.0 - inv2 * (1.0 / 132.0))))))
    return out


def kernel(X, y):
    global _PROGRAM, LAST_RESULTS
    from concourse.bass_utils import run_bass_kernel_spmd
    import concourse.bass_utils as bass_utils

    # artifact upload is not available (nor wanted) in this sandbox; tracing
    # only needs the local NTFF files
    bass_utils.upload_artifacts = lambda tmpdir: "local://" + str(tmpdir)
    _ensure_ntff_hook()

    X = np.asarray(X, dtype=np.float32)
    y = np.asarray(y, dtype=np.int32)
    # sort points by class so each 128-row tile spans <=2 adjacent classes
    # and all same-class columns sit in one contiguous window
    perm = np.argsort(y, kind="stable")
    X = X[perm]
    y = y[perm]

    if _PROGRAM is None:
        _PROGRAM = _build_program()
    nc = _PROGRAM

    sq = np.einsum("nd,nd->n", X.astype(np.float64), X.astype(np.float64))
    oh = (KAPPA * (y[:, None] == np.arange(C)[None, :])).astype(np.float32)

    import ml_dtypes
    B = np.ascontiguousarray(X.T).astype(ml_dtypes.bfloat16)
    negh = (-0.5 * sq).astype(np.float32)
    negh_hi = negh.astype(ml_dtypes.bfloat16)
    negh_lo = (negh - negh_hi.astype(np.float32)).astype(ml_dtypes.bfloat16)
    B2 = np.empty((KX, N), dtype=ml_dtypes.bfloat16)
    B2[:C] = oh.T.astype(ml_dtypes.bfloat16)
    B2[C] = negh_hi
    B2[C + 1] = negh_lo

    counts_per_class = np.bincount(y, minlength=C)
    class_start = np.concatenate([[0], np.cumsum(counts_per_class)])
    wins = np.empty(N // 128, dtype=np.int32)
    for t in range(N // 128):
        c_first = y[128 * t]
        c_last = y[128 * t + 127]
        w = min(int(class_start[c_first]), N - WIN)
        assert int(class_start[c_last + 1]) <= w + WIN
        wins[t] = w

    in_maps = []
    for c in range(NCORES):
        cols = slice(c * ROWS_PER_CORE, (c + 1) * ROWS_PER_CORE)
        A = np.ascontiguousarray(X.T[:, cols]).astype(ml_dtypes.bfloat16)
        A2 = np.empty((KX, ROWS_PER_CORE), dtype=ml_dtypes.bfloat16)
        A2[:C] = oh.T[:, cols].astype(ml_dtypes.bfloat16)
        A2[C] = 1.0
        A2[C + 1] = 1.0
        win = wins[c * RT:(c + 1) * RT].reshape(1, RT)
        in_maps.append({"rhsB": B, "lhsA": A, "rhsB2": B2, "lhsA2": A2,
                        "win": win})

    res = run_bass_kernel_spmd(nc, in_maps, core_ids=list(range(NCORES)))
    LAST_RESULTS = res

    cnt = np.concatenate([r["cnt"].T.reshape(-1) for r in res.results])
    sgn = np.concatenate([r["sgn"].T.reshape(-1) for r in res.results])

    # C_all = (#cols > T') = DVE count + ScalarE sign-sum count
    act_cols = np.full(N, ACT_COLS, dtype=np.float64)
    for c in range(NCORES):
        act_cols[c * ROWS_PER_CORE + ROWS_PER_CORE - 128:
                 (c + 1) * ROWS_PER_CORE] = N - DVE_COLS_LAST
    c_all = cnt + 0.5 * (act_cols + sgn)
    n_per_class = np.bincount(y, minlength=C).astype(np.float64)
    n_same = n_per_class[y]
    m = 2.0 + c_all - n_same

    avg_m = _digamma(m + 1e-7).mean()
    n_x = n_per_class
    avg_n_x = float(np.sum((n_x / N) * _digamma(n_x)))
    mi = (_digamma(np.float64(N))[0] - avg_n_x
          + _digamma(np.float64(KNN))[0] - avg_m)
    mi = float(mi) / np.log(2.0)
    return np.float32(max(mi, 0.0))


# revision 36
# speedup vs baseline: 1.0424x; 1.0424x over previous
"""Trainium2 kernel for nn_DiffClusterMIST (KNN mutual-information estimator).

Math: for each row i of X [8192, 256] with class label y_i:
  d2_ij = |x_i|^2 + |x_j|^2 - 2 x_i.x_j            (squared distance)
  anchor_i = 4th-smallest d2 among same-class j (self included)
  m_i = #{j: d2_ij < anchor_i} - 1                 (over ALL j)
  mi = (psi(N) - sum_c (N_c/N) psi(N_c) + psi(3) - mean_i psi(m_i + 1e-7)) / ln 2
  out = relu(mi)

Device trick: let u_ij = x_i.x_j - |x_j|^2/2. Then d2_ij = |x_i|^2 - 2 u_ij,
so d2-ordering/comparisons within a row are reversed u-orderings - no sqrt,
no per-row |x_i|^2 term, no clamp needed. Class masking is folded into the
matmul: append kappa*onehot(y) features to both operands (adds kappa^2 to
same-class u), plus a (ones, -|x_j|^2/2) feature pair so the matmul directly
produces u'_ij = u_ij + kappa^2*[y_i=y_j] in PSUM:
  A_i = [x_i, kappa*oh(y_i), 1]  (lhsT side),  B_j = [x_j, kappa*oh(y_j), -h_j]
With kappa^2 >> range(u), the row-wise top-4 of u' are exactly the 4 nearest
same-class points; threshold T' = max8(u')[3] - kappa^2 counts all-j strictly
above the anchor:  C_all = n_same(y_i) + #{diff j above}  =>
  m_i = 2 + C_all - n_same(y_i).

Sharding: rows split across the 8 cores (1024 rows each); B replicated.
Each core: 8 row-tiles x 16 col-chunks of fp32r matmul (K=267) -> PSUM,
ScalarE evacuates PSUM->SBUF, DVE max8 per row-tile, then the count is
column-split between DVE (tensor_scalar is_gt + accum) and ScalarE
(Sign activation + accum). Host finishes with exact digammas.
"""
import sys

if "/opt/trn_rl_repo" not in sys.path:
    sys.path.insert(0, "/opt/trn_rl_repo")

import numpy as np

N = 8192
D = 256
C = 10
KNN = 3
KAPPA = 256.0
KAPPA2 = KAPPA * KAPPA
NCORES = 8
ROWS_PER_CORE = N // NCORES          # 1024
RT = ROWS_PER_CORE // 128            # 8 row-tiles per core
KDIM = D + C + 1                     # 267
KX = C + 2                           # packed extra features: onehot, -h hi/lo
CHUNK = 512                          # matmul free dim (one PSUM bank)
GROUP = 2048                         # evac group (4 banks)
NGROUP = N // GROUP                  # 4
DVE_COLS = 7680
DVE_COLS_LAST = 4096                 # balance the post-matmul tail                    # count cols on DVE; rest on ScalarE
ACT_COLS = N - DVE_COLS
WIN = 2048                           # runtime-windowed max8 scan width

_PROGRAM = None
LAST_RESULTS = None


def _ensure_ntff_hook():
    """The agent image's `antenv` lacks `axon_hooks`, which bass_utils
    needs to capture NTFF profiles under axon (trace=True). Provide the
    module and register the ctypes-based hook from trn_agent_boot."""
    import types
    try:
        import antenv.axon_hooks  # noqa: F401
        return
    except ImportError:
        pass
    try:
        import antenv
        from trn_agent_boot.trn_boot import _ntff_profile_via_ctypes
        holder = [None]
        mod = types.ModuleType("antenv.axon_hooks")
        mod.set_axon_ntff_profile_hook = lambda h: holder.__setitem__(0, h)
        mod.get_axon_ntff_profile_hook = lambda: holder[0]
        sys.modules["antenv.axon_hooks"] = mod
        antenv.axon_hooks = mod
        mod.set_axon_ntff_profile_hook(
            _ntff_profile_via_ctypes("/opt/axon/libaxon_pjrt.so"))
    except Exception:
        pass


def _split_multi_waits(nc, mybir):
    """This walrus build allows one sync-wait per instruction; move extra
    waits onto preceding engine-local NoOps (equivalent: streams are
    in-order per engine)."""
    cnt = 0
    for f in nc.m.functions:
        for bb in f.blocks:
            out = []
            changed = False
            for inst in bb.instructions:
                si = inst.sync_info
                waits = list(si.on_wait) if si is not None else []
                if len(waits) > 1:
                    changed = True
                    for w in waits[:-1]:
                        cnt += 1
                        out.append(mybir.InstNoOp(
                            name=f"wsplit-{cnt}",
                            engine=inst.engine,
                            bass_nofuse=True,
                            sync_info=mybir.SyncInfo(on_wait=[w], on_update=[]),
                        ))
                    inst.sync_info = mybir.SyncInfo(
                        on_wait=[waits[-1]], on_update=list(si.on_update))
                out.append(inst)
            if changed:
                bb.instructions = out


def _build_program():
    import concourse.bass as bass
    import concourse.mybir as mybir
    from concourse.bass import ds
    from concourse.expressions import make_scalar_value
    from concourse.tile import TileContext

    f32 = mybir.dt.float32
    f32r = mybir.dt.float32r
    i8 = mybir.dt.int8

    nc = bass.Bass(trn_type="TRN2")
    bf16 = mybir.dt.bfloat16
    rhsB_d = nc.dram_tensor("rhsB", [D, N], bf16, kind="ExternalInput")
    lhsA_d = nc.dram_tensor("lhsA", [D, ROWS_PER_CORE], bf16, kind="ExternalInput")
    rhsB2_d = nc.dram_tensor("rhsB2", [KX, N], bf16, kind="ExternalInput")
    lhsA2_d = nc.dram_tensor("lhsA2", [KX, ROWS_PER_CORE], bf16, kind="ExternalInput")
    win_d = nc.dram_tensor("win", [1, RT], mybir.dt.int32, kind="ExternalInput")
    cnt_d = nc.dram_tensor("cnt", [128, RT], f32, kind="ExternalOutput")
    sgn_d = nc.dram_tensor("sgn", [128, RT], f32, kind="ExternalOutput")
    thr_d = nc.dram_tensor("thr", [128, RT], f32, kind="ExternalOutput")

    KT = [(0, 128), (128, 128)]  # X k-tiles; extra features packed in bf16

    with TileContext(nc) as tc:
        with tc.tile_pool(name="weights", bufs=1) as wpool, \
             tc.tile_pool(name="ubuf", bufs=3) as upool, \
             tc.tile_pool(name="junk", bufs=1) as jpool, \
             tc.tile_pool(name="small", bufs=3) as spool, \
             tc.tile_pool(name="psum", bufs=2, space="PSUM") as ppool:

            # A-block (lhsT) for this core's rows - on the gpsimd (SWDGE)
            # queue so it loads in parallel with the B panels below.
            # The 11 extra feature rows (onehot, ones/-h) are replicated at
            # partition offsets 0/32/64/96 so four chunks' K=11 matmuls can
            # run concurrently in disjoint PE row-groups (tile_position).
            lk = []
            for k, (ks, ksz) in enumerate(KT):
                t = wpool.tile([ksz, ROWS_PER_CORE], bf16, tag=f"lk{k}")
                nc.gpsimd.dma_start(t[:], lhsA_d[ks:ks + ksz, :])
                lk.append(t)
            lb2 = wpool.tile([64 + KX, ROWS_PER_CORE], bf16, tag="lb2")
            rb2 = wpool.tile([64 + KX, N], bf16, tag="rb2")
            for j in range(3):
                nc.gpsimd.dma_start(lb2[32 * j:32 * j + KX, :], lhsA2_d[:, :])
                nc.gpsimd.dma_start(rb2[32 * j:32 * j + KX, :], rhsB2_d[:, :])
            win_sb = wpool.tile([1, RT], mybir.dt.int32, tag="win")
            nc.gpsimd.dma_start(win_sb[:], win_d[:])
            # B (rhs) in 4 column panels per k-tile, loaded in consumption
            # order (panel-major) so the first matmuls start early
            rk = [[None] * NGROUP for _ in KT]
            for j in range(NGROUP):
                for k, (ks, ksz) in enumerate(KT):
                    t = wpool.tile([ksz, GROUP], bf16, tag=f"rk{k}_{j}")
                    nc.sync.dma_start(t[:], rhsB_d[ks:ks + ksz, j * GROUP:(j + 1) * GROUP])
                    rk[k][j] = t

            junkD = jpool.tile([128, DVE_COLS], i8, tag="junkD")
            junkA = jpool.tile([128, N - DVE_COLS_LAST], i8, tag="junkA")
            cntA = jpool.tile([128, RT], f32, tag="cntA")
            sgnA = jpool.tile([128, RT], f32, tag="sgnA")
            thrA = jpool.tile([128, RT], f32, tag="thrA")

            # Software pipeline: tile r's ScalarE sign-count is emitted
            # after tile r+1's matmul/evac section so the ScalarE stream
            # never stalls waiting on DVE's max8 of the same tile.
            pend = None

            def emit_counts(r, u, thr, negthr):
                dcols = DVE_COLS if r < RT - 1 else DVE_COLS_LAST
                nc.vector.tensor_scalar(
                    junkD[:, 0:dcols], u[:, 0:dcols], thr[:], None,
                    op0=mybir.AluOpType.is_gt, op1=mybir.AluOpType.add,
                    accum_out=cntA[:, r:r + 1])
                nc.scalar.activation(
                    junkA[:, 0:N - dcols], u[:, dcols:N],
                    mybir.ActivationFunctionType.Sign,
                    bias=negthr[:], accum_out=sgnA[:, r:r + 1])

            for r in range(RT):
                u = upool.tile([128, N], f32, tag="u")
                for g in range(NGROUP):
                    ps = ppool.tile([128, GROUP], f32, tag="ps")
                    # k-major: consecutive matmuls hit different PSUM banks,
                    # so each bank's accumulation-chain drain is hidden
                    for k in range(2):
                        for c in range(GROUP // CHUNK):
                            lo = c * CHUNK
                            nc.tensor.matmul(
                                ps[:, lo:lo + CHUNK],
                                lk[k][:, r * 128:(r + 1) * 128],
                                rk[k][g][:, lo:lo + CHUNK],
                                start=(k == 0), stop=False)
                    for c in range(GROUP // CHUNK):
                        lo = c * CHUNK
                        po = 32 * (c % 3)
                        nc.tensor.matmul(
                            ps[:, lo:lo + CHUNK],
                            lb2[po:po + KX, r * 128:(r + 1) * 128],
                            rb2[po:po + KX, g * GROUP + lo:g * GROUP + lo + CHUNK],
                            start=False, stop=True,
                            tile_position=(po, 0),
                            skip_group_check=True)
                    nc.scalar.activation(
                        u[:, g * GROUP:(g + 1) * GROUP], ps[:],
                        mybir.ActivationFunctionType.Copy)
                    if g == 3 and pend is not None:
                        emit_counts(*pend)
                        pend = None

                m8 = spool.tile([128, 8], f32, tag="m8")
                wreg = nc.vector.alloc_register(f"win{r}")
                nc.vector.load(wreg, win_sb[0:1, r:r + 1])
                wsv = make_scalar_value(wreg, min_val=0, max_val=N - WIN)
                nc.vector.max(out=m8[:], in_=u[:, ds(wsv, WIN)])
                thr = thrA[:, r:r + 1]
                nc.vector.tensor_scalar_add(thr, m8[:, 3:4], -KAPPA2)
                negthr = spool.tile([128, 1], f32, tag="negthr")
                nc.vector.tensor_scalar_mul(negthr[:], thr, -1.0)
                pend = (r, u, thr, negthr)
            emit_counts(*pend)
            nc.sync.dma_start(cnt_d[:], cntA[:])
            nc.sync.dma_start(sgn_d[:], sgnA[:])
            nc.sync.dma_start(thr_d[:], thrA[:])

    _split_multi_waits(nc, mybir)
    return nc


def _digamma(x):
    """Vectorized digamma, float64, accurate for x > 0."""
    x = np.atleast_1d(np.asarray(x, dtype=np.float64)).copy()
    out = np.zeros_like(x)
    # recurrence psi(x) = psi(x+1) - 1/x until x >= 6
    for _ in range(8):
        mask = x < 6.0
        if not mask.any():
            break
        out[mask] -= 1.0 / x[mask]
        x[mask] += 1.0
    inv = 1.0 / x
    inv2 = inv * inv
    out += (np.log(x) - 0.5 * inv
            - inv2 * (1.0 / 12.0 - inv2 * (1.0 / 120.0 - inv2 * (1.0 / 252.0
                      - inv2 * (1.0 / 240.0 - inv2 * (1.0 / 132.0))))))
    return out


def kernel(X, y):
    global _PROGRAM, LAST_RESULTS
    from concourse.bass_utils import run_bass_kernel_spmd
    import concourse.bass_utils as bass_utils

    # artifact upload is not available (nor wanted) in this sandbox; tracing
    # only needs the local NTFF files
    bass_utils.upload_artifacts = lambda tmpdir: "local://" + str(tmpdir)
    _ensure_ntff_hook()

    X = np.asarray(X, dtype=np.float32)
    y = np.asarray(y, dtype=np.int32)
    # sort points by class so each 128-row tile spans <=2 adjacent classes
    # and all same-class columns sit in one contiguous window
    perm = np.argsort(y, kind="stable")
    X = X[perm]
    y = y[perm]

    if _PROGRAM is None:
        _PROGRAM = _build_program()
    nc = _PROGRAM

    sq = np.einsum("nd,nd->n", X.astype(np.float64), X.astype(np.float64))
    oh = (KAPPA * (y[:, None] == np.arange(C)[None, :])).astype(np.float32)

    import ml_dtypes
    B = np.ascontiguousarray(X.T).astype(ml_dtypes.bfloat16)
    negh = (-0.5 * sq).astype(np.float32)
    negh_hi = negh.astype(ml_dtypes.bfloat16)
    negh_lo = (negh - negh_hi.astype(np.float32)).astype(ml_dtypes.bfloat16)
    B2 = np.empty((KX, N), dtype=ml_dtypes.bfloat16)
    B2[:C] = oh.T.astype(ml_dtypes.bfloat16)
    B2[C] = negh_hi
    B2[C + 1] = negh_lo

    counts_per_class = np.bincount(y, minlength=C)
    class_start = np.concatenate([[0], np.cumsum(counts_per_class)])
    wins = np.empty(N // 128, dtype=np.int32)
    for t in range(N // 128):
        c_first = y[128 * t]
        c_last = y[128 * t + 127]
        w = min(int(class_start[c_first]), N - WIN)
        assert int(class_start[c_last + 1]) <= w + WIN
        wins[t] = w

    in_maps = []
    for c in range(NCORES):
        cols = slice(c * ROWS_PER_CORE, (c + 1) * ROWS_PER_CORE)
        A = np.ascontiguousarray(X.T[:, cols]).astype(ml_dtypes.bfloat16)
        A2 = np.empty((KX, ROWS_PER_CORE), dtype=ml_dtypes.bfloat16)
        A2[:C] = oh.T[:, cols].astype(ml_dtypes.bfloat16)
        A2[C] = 1.0
        A2[C + 1] = 1.0
        win = wins[c * RT:(c + 1) * RT].reshape(1, RT)
        in_maps.append({"rhsB": B, "lhsA": A, "rhsB2": B2, "lhsA2": A2,
                        "win": win})

    res = run_bass_kernel_spmd(nc, in_maps, core_ids=list(range(NCORES)))
    LAST_RESULTS = res

    cnt = np.concatenate([r["cnt"].T.reshape(-1) for r in res.results])
    sgn = np.concatenate([r["sgn"].T.reshape(-1) for r in res.results])

    # C_all = (#cols > T') = DVE count + ScalarE sign-sum count
    act_cols = np.full(N, ACT_COLS, dtype=np.float64)
    for c in range(NCORES):
        act_cols[c * ROWS_PER_CORE + ROWS_PER_CORE - 128:
                 (c + 1) * ROWS_PER_CORE] = N - DVE_COLS_LAST
    c_all = cnt + 0.5 * (act_cols + sgn)
    n_per_class = np.bincount(y, minlength=C).astype(np.float64)
    n_same = n_per_class[y]
    m = 2.0 + c_all - n_same

    avg_m = _digamma(m + 1e-7).mean()
    n_x = n_per_class
    avg_n_x = float(np.sum((n_x / N) * _digamma(n_x)))
    mi = (_digamma(np.float64(N))[0] - avg_n_x
          + _digamma(np.float64(KNN))[0] - avg_m)
    mi = float(mi) / np.log(2.0)
    return np.float32(max(mi, 0.0))


# revision 37
# speedup vs baseline: 1.0567x; 1.0137x over previous
"""Trainium2 kernel for nn_DiffClusterMIST (KNN mutual-information estimator).

Math: for each row i of X [8192, 256] with class label y_i:
  d2_ij = |x_i|^2 + |x_j|^2 - 2 x_i.x_j            (squared distance)
  anchor_i = 4th-smallest d2 among same-class j (self included)
  m_i = #{j: d2_ij < anchor_i} - 1                 (over ALL j)
  mi = (psi(N) - sum_c (N_c/N) psi(N_c) + psi(3) - mean_i psi(m_i + 1e-7)) / ln 2
  out = relu(mi)

Device trick: let u_ij = x_i.x_j - |x_j|^2/2. Then d2_ij = |x_i|^2 - 2 u_ij,
so d2-ordering/comparisons within a row are reversed u-orderings - no sqrt,
no per-row |x_i|^2 term, no clamp needed. Class masking is folded into the
matmul: append kappa*onehot(y) features to both operands (adds kappa^2 to
same-class u), plus a (ones, -|x_j|^2/2) feature pair so the matmul directly
produces u'_ij = u_ij + kappa^2*[y_i=y_j] in PSUM:
  A_i = [x_i, kappa*oh(y_i), 1]  (lhsT side),  B_j = [x_j, kappa*oh(y_j), -h_j]
With kappa^2 >> range(u), the row-wise top-4 of u' are exactly the 4 nearest
same-class points; threshold T' = max8(u')[3] - kappa^2 counts all-j strictly
above the anchor:  C_all = n_same(y_i) + #{diff j above}  =>
  m_i = 2 + C_all - n_same(y_i).

Sharding: rows split across the 8 cores (1024 rows each); B replicated.
Each core: 8 row-tiles x 16 col-chunks of fp32r matmul (K=267) -> PSUM,
ScalarE evacuates PSUM->SBUF, DVE max8 per row-tile, then the count is
column-split between DVE (tensor_scalar is_gt + accum) and ScalarE
(Sign activation + accum). Host finishes with exact digammas.
"""
import sys

if "/opt/trn_rl_repo" not in sys.path:
    sys.path.insert(0, "/opt/trn_rl_repo")

import numpy as np

N = 8192
D = 256
C = 10
KNN = 3
KAPPA = 256.0
KAPPA2 = KAPPA * KAPPA
NCORES = 8
ROWS_PER_CORE = N // NCORES          # 1024
RT = ROWS_PER_CORE // 128            # 8 row-tiles per core
KDIM = D + C + 1                     # 267
KX = C + 2                           # packed extra features: onehot, -h hi/lo
CHUNK = 512                          # matmul free dim (one PSUM bank)
GROUP = 2048                         # evac group (4 banks)
NGROUP = N // GROUP                  # 4
DVE_COLS = 7680
DVE_COLS_LAST = 4096                 # balance the post-matmul tail                    # count cols on DVE; rest on ScalarE
ACT_COLS = N - DVE_COLS
WIN = 2048                           # runtime-windowed max8 scan width

_PROGRAM = None
LAST_RESULTS = None


def _ensure_ntff_hook():
    """The agent image's `antenv` lacks `axon_hooks`, which bass_utils
    needs to capture NTFF profiles under axon (trace=True). Provide the
    module and register the ctypes-based hook from trn_agent_boot."""
    import types
    try:
        import antenv.axon_hooks  # noqa: F401
        return
    except ImportError:
        pass
    try:
        import antenv
        from trn_agent_boot.trn_boot import _ntff_profile_via_ctypes
        holder = [None]
        mod = types.ModuleType("antenv.axon_hooks")
        mod.set_axon_ntff_profile_hook = lambda h: holder.__setitem__(0, h)
        mod.get_axon_ntff_profile_hook = lambda: holder[0]
        sys.modules["antenv.axon_hooks"] = mod
        antenv.axon_hooks = mod
        mod.set_axon_ntff_profile_hook(
            _ntff_profile_via_ctypes("/opt/axon/libaxon_pjrt.so"))
    except Exception:
        pass


def _split_multi_waits(nc, mybir):
    """This walrus build allows one sync-wait per instruction; move extra
    waits onto preceding engine-local NoOps (equivalent: streams are
    in-order per engine)."""
    cnt = 0
    for f in nc.m.functions:
        for bb in f.blocks:
            out = []
            changed = False
            for inst in bb.instructions:
                si = inst.sync_info
                waits = list(si.on_wait) if si is not None else []
                if len(waits) > 1:
                    changed = True
                    for w in waits[:-1]:
                        cnt += 1
                        out.append(mybir.InstNoOp(
                            name=f"wsplit-{cnt}",
                            engine=inst.engine,
                            bass_nofuse=True,
                            sync_info=mybir.SyncInfo(on_wait=[w], on_update=[]),
                        ))
                    inst.sync_info = mybir.SyncInfo(
                        on_wait=[waits[-1]], on_update=list(si.on_update))
                out.append(inst)
            if changed:
                bb.instructions = out


def _build_program():
    import concourse.bass as bass
    import concourse.mybir as mybir
    from concourse.bass import ds
    from concourse.expressions import make_scalar_value
    from concourse.tile import TileContext

    f32 = mybir.dt.float32
    f32r = mybir.dt.float32r
    i8 = mybir.dt.int8

    nc = bass.Bass(trn_type="TRN2")
    bf16 = mybir.dt.bfloat16
    rhsB_d = nc.dram_tensor("rhsB", [D, N], bf16, kind="ExternalInput")
    lhsA_d = nc.dram_tensor("lhsA", [D, ROWS_PER_CORE], bf16, kind="ExternalInput")
    rhsB2_d = nc.dram_tensor("rhsB2", [KX, N], bf16, kind="ExternalInput")
    lhsA2_d = nc.dram_tensor("lhsA2", [KX, ROWS_PER_CORE], bf16, kind="ExternalInput")
    win_d = nc.dram_tensor("win", [1, RT], mybir.dt.int32, kind="ExternalInput")
    cnt_d = nc.dram_tensor("cnt", [128, RT], f32, kind="ExternalOutput")
    sgn_d = nc.dram_tensor("sgn", [128, RT], f32, kind="ExternalOutput")
    thr_d = nc.dram_tensor("thr", [128, RT], f32, kind="ExternalOutput")

    KT = [(0, 128), (128, 128)]  # X k-tiles; extra features packed in bf16

    with TileContext(nc) as tc:
        with tc.tile_pool(name="weights", bufs=1) as wpool, \
             tc.tile_pool(name="ubuf", bufs=3) as upool, \
             tc.tile_pool(name="junk", bufs=1) as jpool, \
             tc.tile_pool(name="small", bufs=3) as spool, \
             tc.tile_pool(name="psum", bufs=2, space="PSUM") as ppool:

            # A-block (lhsT) for this core's rows - on the gpsimd (SWDGE)
            # queue so it loads in parallel with the B panels below.
            # The 11 extra feature rows (onehot, ones/-h) are replicated at
            # partition offsets 0/32/64/96 so four chunks' K=11 matmuls can
            # run concurrently in disjoint PE row-groups (tile_position).
            lk = []
            for k, (ks, ksz) in enumerate(KT):
                t = wpool.tile([ksz, ROWS_PER_CORE], bf16, tag=f"lk{k}")
                nc.gpsimd.dma_start(t[:], lhsA_d[ks:ks + ksz, :])
                lk.append(t)
            lb2 = wpool.tile([64 + KX, ROWS_PER_CORE], bf16, tag="lb2")
            rb2 = wpool.tile([64 + KX, N], bf16, tag="rb2")
            for j in range(3):
                nc.gpsimd.dma_start(lb2[32 * j:32 * j + KX, :], lhsA2_d[:, :])
                nc.gpsimd.dma_start(rb2[32 * j:32 * j + KX, :], rhsB2_d[:, :])
            win_sb = wpool.tile([1, RT], mybir.dt.int32, tag="win")
            nc.gpsimd.dma_start(win_sb[:], win_d[:])
            # B (rhs) in 4 column panels per k-tile, loaded in consumption
            # order (panel-major) so the first matmuls start early
            rk = [[None] * NGROUP for _ in KT]
            for j in range(NGROUP):
                for k, (ks, ksz) in enumerate(KT):
                    t = wpool.tile([ksz, GROUP], bf16, tag=f"rk{k}_{j}")
                    base = j * GROUP
                    nc.sync.dma_start(t[:, 0:1024],
                                      rhsB_d[ks:ks + ksz, base:base + 1024])
                    nc.sync.dma_start(t[:, 1024:GROUP],
                                      rhsB_d[ks:ks + ksz, base + 1024:base + GROUP])
                    rk[k][j] = t

            junkD = jpool.tile([128, DVE_COLS], i8, tag="junkD")
            junkA = jpool.tile([128, N - DVE_COLS_LAST], i8, tag="junkA")
            cntA = jpool.tile([128, RT], f32, tag="cntA")
            sgnA = jpool.tile([128, RT], f32, tag="sgnA")
            thrA = jpool.tile([128, RT], f32, tag="thrA")

            # Software pipeline: tile r's ScalarE sign-count is emitted
            # after tile r+1's matmul/evac section so the ScalarE stream
            # never stalls waiting on DVE's max8 of the same tile.
            pend = None

            def emit_counts(r, u, thr, negthr):
                dcols = DVE_COLS if r < RT - 1 else DVE_COLS_LAST
                nc.vector.tensor_scalar(
                    junkD[:, 0:dcols], u[:, 0:dcols], thr[:], None,
                    op0=mybir.AluOpType.is_gt, op1=mybir.AluOpType.add,
                    accum_out=cntA[:, r:r + 1])
                nc.scalar.activation(
                    junkA[:, 0:N - dcols], u[:, dcols:N],
                    mybir.ActivationFunctionType.Sign,
                    bias=negthr[:], accum_out=sgnA[:, r:r + 1])

            for r in range(RT):
                u = upool.tile([128, N], f32, tag="u")
                for g in range(NGROUP):
                    ps = ppool.tile([128, GROUP], f32, tag="ps")
                    # k-major: consecutive matmuls hit different PSUM banks,
                    # so each bank's accumulation-chain drain is hidden
                    for k in range(2):
                        for c in range(GROUP // CHUNK):
                            lo = c * CHUNK
                            nc.tensor.matmul(
                                ps[:, lo:lo + CHUNK],
                                lk[k][:, r * 128:(r + 1) * 128],
                                rk[k][g][:, lo:lo + CHUNK],
                                start=(k == 0), stop=False)
                    for c in range(GROUP // CHUNK):
                        lo = c * CHUNK
                        po = 32 * (c % 3)
                        nc.tensor.matmul(
                            ps[:, lo:lo + CHUNK],
                            lb2[po:po + KX, r * 128:(r + 1) * 128],
                            rb2[po:po + KX, g * GROUP + lo:g * GROUP + lo + CHUNK],
                            start=False, stop=True,
                            tile_position=(po, 0),
                            skip_group_check=True)
                    nc.scalar.activation(
                        u[:, g * GROUP:(g + 1) * GROUP], ps[:],
                        mybir.ActivationFunctionType.Copy)
                    if g == 3 and pend is not None:
                        emit_counts(*pend)
                        pend = None

                m8 = spool.tile([128, 8], f32, tag="m8")
                wreg = nc.vector.alloc_register(f"win{r}")
                nc.vector.load(wreg, win_sb[0:1, r:r + 1])
                wsv = make_scalar_value(wreg, min_val=0, max_val=N - WIN)
                nc.vector.max(out=m8[:], in_=u[:, ds(wsv, WIN)])
                thr = thrA[:, r:r + 1]
                nc.vector.tensor_scalar_add(thr, m8[:, 3:4], -KAPPA2)
                negthr = spool.tile([128, 1], f32, tag="negthr")
                nc.vector.tensor_scalar_mul(negthr[:], thr, -1.0)
                pend = (r, u, thr, negthr)
            emit_counts(*pend)
            nc.sync.dma_start(cnt_d[:], cntA[:])
            nc.sync.dma_start(sgn_d[:], sgnA[:])
            nc.sync.dma_start(thr_d[:], thrA[:])

    _split_multi_waits(nc, mybir)
    return nc


def _digamma(x):
    """Vectorized digamma, float64, accurate for x > 0."""
    x = np.atleast_1d(np.asarray(x, dtype=np.float64)).copy()
    out = np.zeros_like(x)
    # recurrence psi(x) = psi(x+1) - 1/x until x >= 6
    for _ in range(8):
        mask = x < 6.0
        if not mask.any():
            break
        out[mask] -= 1.0 / x[mask]
        x[mask] += 1.0
    inv = 1.0 / x
    inv2 = inv * inv
    out += (np.log(x) - 0.5 * inv
            - inv2 * (1.0 / 12.0 - inv2 * (1.0 / 120.0 - inv2 * (1.0 / 252.0
                      - inv2 * (1.0 / 240.0 - inv2 * (1.0 / 132.0))))))
    return out


def kernel(X, y):
    global _PROGRAM, LAST_RESULTS
    from concourse.bass_utils import run_bass_kernel_spmd
    import concourse.bass_utils as bass_utils

    # artifact upload is not available (nor wanted) in this sandbox; tracing
    # only needs the local NTFF files
    bass_utils.upload_artifacts = lambda tmpdir: "local://" + str(tmpdir)
    _ensure_ntff_hook()

    X = np.asarray(X, dtype=np.float32)
    y = np.asarray(y, dtype=np.int32)
    # sort points by class so each 128-row tile spans <=2 adjacent classes
    # and all same-class columns sit in one contiguous window
    perm = np.argsort(y, kind="stable")
    X = X[perm]
    y = y[perm]

    if _PROGRAM is None:
        _PROGRAM = _build_program()
    nc = _PROGRAM

    sq = np.einsum("nd,nd->n", X.astype(np.float64), X.astype(np.float64))
    oh = (KAPPA * (y[:, None] == np.arange(C)[None, :])).astype(np.float32)

    import ml_dtypes
    B = np.ascontiguousarray(X.T).astype(ml_dtypes.bfloat16)
    negh = (-0.5 * sq).astype(np.float32)
    negh_hi = negh.astype(ml_dtypes.bfloat16)
    negh_lo = (negh - negh_hi.astype(np.float32)).astype(ml_dtypes.bfloat16)
    B2 = np.empty((KX, N), dtype=ml_dtypes.bfloat16)
    B2[:C] = oh.T.astype(ml_dtypes.bfloat16)
    B2[C] = negh_hi
    B2[C + 1] = negh_lo

    counts_per_class = np.bincount(y, minlength=C)
    class_start = np.concatenate([[0], np.cumsum(counts_per_class)])
    wins = np.empty(N // 128, dtype=np.int32)
    for t in range(N // 128):
        c_first = y[128 * t]
        c_last = y[128 * t + 127]
        w = min(int(class_start[c_first]), N - WIN)
        assert int(class_start[c_last + 1]) <= w + WIN
        wins[t] = w

    in_maps = []
    for c in range(NCORES):
        cols = slice(c * ROWS_PER_CORE, (c + 1) * ROWS_PER_CORE)
        A = np.ascontiguousarray(X.T[:, cols]).astype(ml_dtypes.bfloat16)
        A2 = np.empty((KX, ROWS_PER_CORE), dtype=ml_dtypes.bfloat16)
        A2[:C] = oh.T[:, cols].astype(ml_dtypes.bfloat16)
        A2[C] = 1.0
        A2[C + 1] = 1.0
        win = wins[c * RT:(c + 1) * RT].reshape(1, RT)
        in_maps.append({"rhsB": B, "lhsA": A, "rhsB2": B2, "lhsA2": A2,
                        "win": win})

    res = run_bass_kernel_spmd(nc, in_maps, core_ids=list(range(NCORES)))
    LAST_RESULTS = res

    cnt = np.concatenate([r["cnt"].T.reshape(-1) for r in res.results])
    sgn = np.concatenate([r["sgn"].T.reshape(-1) for r in res.results])

    # C_all = (#cols > T') = DVE count + ScalarE sign-sum count
    act_cols = np.full(N, ACT_COLS, dtype=np.float64)
    for c in range(NCORES):
        act_cols[c * ROWS_PER_CORE + ROWS_PER_CORE - 128:
                 (c + 1) * ROWS_PER_CORE] = N - DVE_COLS_LAST
    c_all = cnt + 0.5 * (act_cols + sgn)
    n_per_class = np.bincount(y, minlength=C).astype(np.float64)
    n_same = n_per_class[y]
    m = 2.0 + c_all - n_same

    avg_m = _digamma(m + 1e-7).mean()
    n_x = n_per_class
    avg_n_x = float(np.sum((n_x / N) * _digamma(n_x)))
    mi = (_digamma(np.float64(N))[0] - avg_n_x
          + _digamma(np.float64(KNN))[0] - avg_m)
    mi = float(mi) / np.log(2.0)
    return np.float32(max(mi, 0.0))
